# revision 1
# baseline (speedup 1.0000x reference)
"""Trainium2 Bass kernel for MultiHeadedAttention (B=4, S=2048, H=16, D=1024).

Sharding: 8 cores = 4-way batch DP x 2-way head TP (8 heads per core).
Each core computes, for its batch b and head-group g:
    partial_out[b] = softmax_causal(Q_g K_g^T / 8) V_g  @ Wp.T[g-slice]
Host gathers: out[b] = partial(b,g=0) + partial(b,g=1) + bp + (bv-fold terms).

Per-core device algorithm ("transposed flash"):
  - All activations/weights cast to bf16 on host; X^T (i.e. [D,S]) layouts
    are prepared on host so every DMA is contiguous and every matmul
    contraction sits on the partition axis.
  - Q_t/K_t computed pair-packed: [128(2 heads x 64dk), S] = Wpair^T.T @ X^T.
  - Scores computed TRANSPOSED per (head, k-block): S_t[k,q] so that
    exp(S_t/8) (ACT) directly yields U^T in SBUF, ready as the moving
    operand of the A@V matmul -- no PE/DVE transposes anywhere.
  - A@V uses lhsT = [V | ones] (65 cols): PSUM row 64 accumulates the
    softmax denominator for free.
  - Normalization (fused per q-chunk into the attention stream): sums rows
    staged through a DRAM scratch into [NH,512] tiles, DVE reciprocal,
    selection-matrix PE broadcast, one DVE multiply on Z^T; the output
    projection for that q-chunk follows immediately so it overlaps the
    remaining attention.
  - Output projection from Z^T tiles (lhsT) against host-sliced Wp^T rows.
"""

import os

import numpy as np
import ml_dtypes

import concourse.tile as tile
import concourse.mybir as mybir
from concourse import bacc
from concourse.bass_utils import run_bass_kernel_spmd

BF16 = mybir.dt.bfloat16
F32 = mybir.dt.float32
NPBF16 = ml_dtypes.bfloat16

DK = 64  # head dim (fixed)


def _chunks(start, end, step=512):
    """Yield [a,b) ranges from start to end, split at multiples of `step`."""
    a = start
    while a < end:
        b = min(end, (a // step + 1) * step)
        yield a, b
        a = b


def build_body(tc, out_ap, ins, S, D, NH, causal):
    """Emit the per-core program. ins: dict of dram APs."""
    nc = tc.nc
    nhp = NH // 2          # head pairs
    ND = D // 128          # contraction chunks for projections
    NQB = S // 128         # 128-blocks along seq
    NQC = S // 512         # 512-chunks along seq
    VST = 2 * (DK + 1)     # V2 stride per k-block: [vA(64)|1|vB(64)|1]

    Exp = mybir.ActivationFunctionType.Exp
    mult = mybir.AluOpType.mult

    XE = 2 if S >= 1024 else 1   # x tiles split into XE column groups
    XW = S // XE
    pool = tc.alloc_tile_pool(name="sb", bufs=2)
    psum = tc.alloc_tile_pool(name="ps", bufs=1, space="PSUM")

    # ---- constants ----
    triu = pool.tile([128, 128], BF16, name="triu", tag="triu", bufs=1)
    nc.vector.memset(triu, 1.0)
    if causal:
        # keep where (col - part) >= 0, else 0  -> upper-tri incl diagonal
        nc.gpsimd.affine_select(
            out=triu, in_=triu, compare_op=mybir.AluOpType.is_ge,
            fill=0.0, base=0, pattern=[[1, 128]], channel_multiplier=-1,
        )
    selb = pool.tile([NH, nhp * 128], BF16, name="selb", tag="selb", bufs=1)

    # ---- persistent tiles ----
    qt = [pool.tile([128, S], BF16, name=f"qt{p}", tag=f"qt{p}", bufs=1) for p in range(nhp)]
    kt = [pool.tile([128, S], BF16, name=f"kt{p}", tag=f"kt{p}", bufs=1) for p in range(nhp)]
    v2 = [pool.tile([128, NQB * VST], BF16, name=f"v2{p}", tag=f"v2{p}", bufs=1) for p in range(nhp)]
    z2 = [pool.tile([128, S], BF16, name=f"z2{p}", tag=f"z2{p}", bufs=1) for p in range(nhp)]
    wp = [pool.tile([128, D], BF16, name=f"wp{p}", tag=f"wp{p}", bufs=1) for p in range(nhp)]
    bqs = [pool.tile([128, 1], F32, name=f"bq{p}", tag=f"bq{p}", bufs=1) for p in range(nhp)]
    bks = [pool.tile([128, 1], F32, name=f"bk{p}", tag=f"bk{p}", bufs=1) for p in range(nhp)]
    NSUM = NQC * NH
    dram = tc.alloc_tile_pool(name="dr", bufs=1, space="DRAM")
    sums_scr = dram.tile([NSUM, 512], F32, name="sums_scr", tag="sums_scr",
                         bufs=1)

    for p in range(nhp):
        nc.sync.dma_start(bqs[p], ins["bq2"][p])
        nc.sync.dma_start(bks[p], ins["bk2"][p])

    def normalize_and_outproj(qc):
        """Normalize all pairs' Z^T for this q-chunk and emit its out-proj.
        Called as soon as the last pair finishes the chunk, so this work
        overlaps the remaining attention on all engines."""
        sums_t = pool.tile([NH, 512], F32, name="sumq", tag="sumq", bufs=1)
        nc.sync.dma_start(sums_t, sums_scr[qc * NH:(qc + 1) * NH, :])
        recip_t = pool.tile([NH, 512], F32, name="recq", tag="recq", bufs=1)
        # sums are softmax denominators in [1, ~S]: approx_fast's 51-ULP
        # error is far below the bf16 rounding applied right after.
        nc.vector.reciprocal_approx_fast(recip_t, sums_t)
        recip_b = pool.tile([NH, 512], BF16, name="recb", tag="recb", bufs=1)
        nc.vector.tensor_copy(recip_b, recip_t)
        for p in range(nhp):
            bc = psum.tile([128, 512], F32, name="pw", tag="pw", bufs=2)
            nc.tensor.matmul(bc, selb[:, p * 128:(p + 1) * 128], recip_b,
                             start=True, stop=True)
            nc.vector.tensor_tensor(
                z2[p][:, qc * 512:(qc + 1) * 512],
                z2[p][:, qc * 512:(qc + 1) * 512], bc, mult)
        for qb in range(4 * qc, 4 * qc + 4):
            for oa, ob in _chunks(0, D):
                ps = psum.tile([128, ob - oa], F32, name="pw", tag="pw",
                               bufs=2)
                for p in range(nhp):
                    nc.tensor.matmul(
                        ps, z2[p][:, qb * 128:(qb + 1) * 128],
                        wp[p][:, oa:ob],
                        start=(p == 0), stop=(p == nhp - 1),
                    )
                ot = pool.tile([128, ob - oa], BF16, name="o", tag="o",
                               bufs=2)
                nc.vector.tensor_copy(ot, ps)
                nc.sync.dma_start(
                    out_ap[qb * 128:(qb + 1) * 128, oa:ob], ot)

    # ---- phase 1a: Q_t / K_t projections (pair-packed) ----
    for name, xin, win, bias_sb, out_sb in (
        ("q", ins["xqT"], ins["wq2"], bqs, qt),
        ("k", ins["xkT"], ins["wk2"], bks, kt),
    ):
        # pair-0 weights load BEFORE the big x streams so the very first
        # matmul only waits for x chunk 0, not the whole input queue.
        ws0 = []
        for d in range(ND):
            wt = pool.tile([128, 128], BF16, name="w", tag="w", bufs=ND)
            nc.sync.dma_start(wt, win[0, d])
            ws0.append(wt)
        # e-major emission: the qc=0 matmuls need only the e=0 halves,
        # so they must be first in the DMA queue.
        xs = [[None] * XE for _ in range(ND)]
        for e in range(XE):
            for d in range(ND):
                xt = pool.tile([128, XW], BF16, name="x", tag="x",
                               bufs=19)
                nc.sync.dma_start(
                    xt, xin[d * 128:(d + 1) * 128, e * XW:(e + 1) * XW])
                xs[d][e] = xt
        for p in range(nhp):
            if p == 0:
                ws = ws0
            else:
                ws = []
                for d in range(ND):
                    wt = pool.tile([128, 128], BF16, name="w", tag="w",
                                   bufs=ND)
                    nc.sync.dma_start(wt, win[p, d])
                    ws.append(wt)
            for qc in range(NQC):
                ps = psum.tile([128, 512], F32, name="pw", tag="pw", bufs=2)
                for d in range(ND):
                    e, eo = divmod(qc * 512, XW)
                    nc.tensor.matmul(
                        ps, ws[d], xs[d][e][:, eo:eo + 512],
                        start=(d == 0), stop=(d == ND - 1),
                    )
                nc.vector.tensor_scalar_add(
                    out_sb[p][:, qc * 512:(qc + 1) * 512], ps, bias_sb[p])

    # wp/selb are needed only by the fused normalize/out-proj; load them
    # after the projection streams so they don't delay xq/xk.
    nc.sync.dma_start(selb, ins["selb"])
    for p in range(nhp):
        nc.sync.dma_start(wp[p], ins["wpT"][p])

    # ---- phase 1b + 2: V projections + attention, software-pipelined ----
    # Pair p's attention (ACT-heavy) is interleaved at emission time with
    # pair p+1's V-projection blocks (PE-heavy) and with already-available
    # AV matmuls, so the in-order PE stream always has ready work while ACT
    # chews through the exps.
    xs = [[None] * XE for _ in range(ND)]
    for e in range(XE):
        for d in range(ND):
            xt = pool.tile([128, XW], BF16, name="x", tag="x",
                           bufs=19)
            nc.sync.dma_start(
                xt, ins["xvT"][d * 128:(d + 1) * 128, e * XW:(e + 1) * XW])
            xs[d][e] = xt

    vws = {}

    def prep_vproj(p):
        nc.vector.memset(v2[p], 1.0)  # ones columns survive at 64 and 129
        ws = []
        for d in range(ND):
            wt = pool.tile([128, 128], BF16, name="w", tag="w", bufs=ND)
            nc.sync.dma_start(wt, ins["wv2"][p, d])
            ws.append(wt)
        vws[p] = ws

    def emit_vproj_block(p, sb):
        ps = psum.tile([128, 128], F32, name="pw", tag="pw", bufs=2)
        for d in range(ND):
            e, eo = divmod(sb * 128, XW)
            nc.tensor.matmul(
                ps, xs[d][e][:, eo:eo + 128], vws[p][d],
                start=(d == 0), stop=(d == ND - 1),
            )
        dst = v2[p][:, sb * VST: sb * VST + VST].rearrange(
            "p (a b) -> p a b", a=2)[:, :, 0:DK]
        nc.vector.tensor_copy(dst, ps.rearrange("p (a b) -> p a b", a=2))

    prep_vproj(0)
    for sb in range(NQB):
        emit_vproj_block(0, sb)

    for p in range(nhp):
        if p + 1 < nhp:
            prep_vproj(p + 1)
            vfill = [(p + 1, sb) for sb in range(NQB)]
        else:
            vfill = []
        if not causal:
            # memory-lean fallback for arbitrary masks: recompute each
            # (j, qc) score window instead of caching U tiles across qc.
            for qc in range(NQC):
                ztg = [psum.tile([65, 512], F32, name=f"z{half}",
                                 tag=f"z{half}", bufs=1)
                       for half in range(2)]
                for j in range(NQB):
                    mk = pool.tile([128, 512], BF16, name="mk", tag="mk",
                                   bufs=2)
                    nc.sync.dma_start(
                        mk, ins["maskT"][j * 128:(j + 1) * 128,
                                         qc * 512:(qc + 1) * 512])
                    ug = []
                    for half in range(2):
                        po = half * 64
                        st = psum.tile([128, 512], F32, name="s", tag="s",
                                       bufs=2)
                        nc.tensor.matmul(
                            st, kt[p][po:po + 64, j * 128:(j + 1) * 128],
                            qt[p][po:po + 64, qc * 512:(qc + 1) * 512],
                            start=True, stop=True)
                        uu = pool.tile([128, 512], BF16, name=f"ug{half}",
                                       tag=f"ug{half}", bufs=2)
                        nc.scalar.activation(uu, st, Exp, scale=0.125)
                        nc.vector.tensor_tensor(uu, uu, mk, mult)
                        ug.append(uu)
                    for half in range(2):
                        nc.tensor.matmul(
                            ztg[half],
                            v2[p][:, j * VST + half * (DK + 1):
                                  j * VST + half * (DK + 1) + DK + 1],
                            ug[half],
                            start=(j == 0), stop=(j == NQB - 1))
                for half in range(2):
                    r = qc * NH + 2 * p + half
                    srow = pool.tile([1, 512], F32, name="srow", tag="srow",
                                     bufs=2)
                    nc.vector.tensor_copy(srow, ztg[half][64:65, :])
                    nc.sync.dma_start(sums_scr[r:r + 1, :], srow)
                    nc.vector.tensor_copy(
                        z2[p][half * 64:half * 64 + 64,
                              qc * 512:(qc + 1) * 512], ztg[half][0:64, :])
                if p == nhp - 1:
                    normalize_and_outproj(qc)
            while vfill:
                emit_vproj_block(*vfill.pop(0))
            continue

        utiles = {}
        for qc in range(NQC):
            jmax = 4 * qc + 3
            fresh_js = [j for j in range(jmax + 1) if j // 4 == qc]
            old_js = [j for j in range(jmax + 1) if j not in fresh_js]
            zts = [psum.tile([65, 512], F32, name=f"z{half}",
                             tag=f"z{half}", bufs=1) for half in range(2)]

            def emit_av(j, last):
                us, base_q, off = utiles[j]
                aoff = 512 * qc - base_q
                zoff = max(off - aoff, 0)
                for half in range(2):
                    nc.tensor.matmul(
                        zts[half][:, zoff:512],
                        v2[p][:, j * VST + half * (DK + 1):
                              j * VST + half * (DK + 1) + DK + 1],
                        us[half][:, aoff + zoff: aoff + 512],
                        start=(j == 0), stop=last,
                    )

            ready = list(old_js)      # AVs whose U data is available
            emitted = []

            def pop_filler():
                # never emit the group-closing AV here: the drain loop below
                # owns the stop=True flag.
                if ready and len(emitted) < jmax:
                    j_ = ready.pop(0)
                    emit_av(j_, last=False)
                    emitted.append(j_)
                elif vfill:
                    emit_vproj_block(*vfill.pop(0))

            for j in fresh_js:
                base_q = 512 * (j // 4)
                W = S - base_q
                off = 128 * j - base_q
                us = [pool.tile([128, W], BF16, name=f"u{half}_{j}",
                                tag=f"u{half}_{j}", bufs=1)
                      for half in range(2)]
                utiles[j] = (us, base_q, off)
                for w0 in range(0, W, 1024):
                    w1 = min(w0 + 1024, W)
                    if w1 <= off:
                        continue
                    lo = max(off - w0, 0)
                    sts = []
                    for half in range(2):
                        po = half * 64
                        st = psum.tile([128, min(1024, W - w0)], F32,
                                       name="s", tag="s", bufs=2)
                        for a, b in _chunks(lo, w1 - w0):
                            nc.tensor.matmul(
                                st[:, a:b],
                                kt[p][po:po + 64, j * 128:(j + 1) * 128],
                                qt[p][po:po + 64,
                                      base_q + w0 + a: base_q + w0 + b],
                                start=True, stop=True,
                            )
                        sts.append(st)
                    for half in range(2):
                        nc.scalar.activation(
                            us[half][:, w0 + lo:w1],
                            sts[half][:, lo:w1 - w0], Exp, scale=0.125)
                    if w0 <= off:
                        # diag block lives in the first valid window: mask
                        # now so this j's own AV unblocks without waiting
                        # for the remaining windows.
                        for half in range(2):
                            nc.vector.tensor_tensor(
                                us[half][:, off:off + 128],
                                us[half][:, off:off + 128], triu, mult)
                        # this j's qc-window AV only reads window 0 -> ready
                        ready.append(j)
                    pop_filler()
            while ready:
                j_ = ready.pop(0)
                emitted.append(j_)
                emit_av(j_, last=(len(emitted) == jmax + 1))
            for half in range(2):
                r = qc * NH + 2 * p + half
                srow = pool.tile([1, 512], F32, name="srow", tag="srow",
                                 bufs=2)
                nc.vector.tensor_copy(srow, zts[half][64:65, :])
                nc.sync.dma_start(sums_scr[r:r + 1, :], srow)
                nc.vector.tensor_copy(
                    z2[p][half * 64:half * 64 + 64,
                          qc * 512:(qc + 1) * 512], zts[half][0:64, :])
            if p == nhp - 1:
                normalize_and_outproj(qc)
        while vfill:
            emit_vproj_block(*vfill.pop(0))

    pool.release()
    psum.release()
    dram.release()


def build_program(S, D, NH, causal, num_devices):
    nc = bacc.Bacc("TRN2", target_bir_lowering=False, debug=False,
                   num_devices=num_devices)
    nhp = NH // 2
    ND = D // 128
    ins = {
        "xqT": nc.dram_tensor("xqT", [D, S], BF16, kind="ExternalInput").ap(),
        "xkT": nc.dram_tensor("xkT", [D, S], BF16, kind="ExternalInput").ap(),
        "xvT": nc.dram_tensor("xvT", [D, S], BF16, kind="ExternalInput").ap(),
        "wq2": nc.dram_tensor("wq2", [nhp, ND, 128, 128], BF16, kind="ExternalInput").ap(),
        "wk2": nc.dram_tensor("wk2", [nhp, ND, 128, 128], BF16, kind="ExternalInput").ap(),
        "wv2": nc.dram_tensor("wv2", [nhp, ND, 128, 128], BF16, kind="ExternalInput").ap(),
        "bq2": nc.dram_tensor("bq2", [nhp, 128, 1], F32, kind="ExternalInput").ap(),
        "bk2": nc.dram_tensor("bk2", [nhp, 128, 1], F32, kind="ExternalInput").ap(),
        "wpT": nc.dram_tensor("wpT", [nhp, 128, D], BF16, kind="ExternalInput").ap(),
        "selb": nc.dram_tensor("selb", [NH, (NH // 2) * 128], BF16,
                               kind="ExternalInput").ap(),
    }
    if not causal:
        ins["maskT"] = nc.dram_tensor("maskT", [S, S], BF16,
                                      kind="ExternalInput").ap()
    out_ap = nc.dram_tensor("out", [S, D], BF16, kind="ExternalOutput").ap()
    with tile.TileContext(nc) as tc:
        build_body(tc, out_ap, ins, S, D, NH, causal)
    nc.compile()
    return nc


def _prep_core_weights(Wq, bq, Wk, bk, Wv, Wp, g, NH):
    """Host-side weight shard/transpose for head-group g (NH heads)."""
    nhp = NH // 2
    D = Wq.shape[2]
    ND = D // 128
    out = {}
    for nm, W in (("wq2", Wq), ("wk2", Wk), ("wv2", Wv)):
        t = np.empty((nhp, ND, 128, 128), NPBF16)
        for p in range(nhp):
            hA = g * NH + 2 * p
            for d in range(ND):
                t[p, d, :, 0:DK] = W[hA][:, d * 128:(d + 1) * 128].T
                t[p, d, :, DK:128] = W[hA + 1][:, d * 128:(d + 1) * 128].T
        out[nm] = t
    for nm, b in (("bq2", bq), ("bk2", bk)):
        t = np.empty((nhp, 128, 1), np.float32)
        for p in range(nhp):
            hA = g * NH + 2 * p
            t[p, 0:DK, 0] = b[hA]
            t[p, DK:128, 0] = b[hA + 1]
        out[nm] = t
    # Wp.T rows for this group's concat-features, pair-chunked
    WpT = np.ascontiguousarray(Wp.T[g * NH * DK:(g + 1) * NH * DK, :])
    out["wpT"] = WpT.reshape(nhp, 128, D).astype(NPBF16)
    return out


def _make_selb(S, NH):
    """Selection matrix for the per-qc recip broadcast: [NH, nhp*128];
    column block p, column m picks sums row 2p + (m>=64)."""
    nhp = NH // 2
    selb = np.zeros((NH, nhp * 128), NPBF16)
    for p in range(nhp):
        selb[2 * p, p * 128: p * 128 + DK] = 1.0
        selb[2 * p + 1, p * 128 + DK: p * 128 + 128] = 1.0
    return selb


def kernel(**inputs):
    B, S, H, D = 4, 2048, 16, 1024
    NH = H // 2  # heads per core (2-way head TP)
    q = np.asarray(inputs["query"], np.float32)
    k = np.asarray(inputs["key"], np.float32)
    v = np.asarray(inputs["value"], np.float32)
    Wq = np.asarray(inputs["Wq"], np.float32)
    bq = np.asarray(inputs["bq"], np.float32)
    Wk = np.asarray(inputs["Wk"], np.float32)
    bk = np.asarray(inputs["bk"], np.float32)
    Wv = np.asarray(inputs["Wv"], np.float32)
    bv = np.asarray(inputs["bv"], np.float32)
    Wp = np.asarray(inputs["Wp"], np.float32)
    bp = np.asarray(inputs["bp"], np.float32)
    mask = np.asarray(inputs["mask"])

    tril = np.tril(np.ones((S, S), dtype=bool))
    causal = all(np.array_equal(mask[b], tril) for b in range(B))

    # per-batch transposed activations (shared by the 2 cores of a batch)
    xT = {}
    for b in range(B):
        xT[b] = (
            np.ascontiguousarray(q[b].T).astype(NPBF16),
            np.ascontiguousarray(k[b].T).astype(NPBF16),
            np.ascontiguousarray(v[b].T).astype(NPBF16),
        )
    gw = [_prep_core_weights(Wq, bq, Wk, bk, Wv, Wp, g, NH) for g in range(2)]
    mT = None
    if not causal:
        mT = [np.ascontiguousarray(mask[b].T).astype(NPBF16) for b in range(B)]

    selb = _make_selb(S, NH)
    in_maps = []
    for c in range(8):
        b, g = c // 2, c % 2
        m = {"xqT": xT[b][0], "xkT": xT[b][1], "xvT": xT[b][2],
             "selb": selb}
        m.update(gw[g])
        if not causal:
            m["maskT"] = mT[b]
        in_maps.append(m)

    nc = build_program(S, D, NH, causal, num_devices=8)
    trace = bool(int(os.environ.get("KERNEL_TRACE", "0")))
    try:
        res = run_bass_kernel_spmd(nc, in_maps, core_ids=list(range(8)),
                                   trace=trace)
    except ModuleNotFoundError:
        # NTFF profiling hook unavailable on this client; run untraced.
        res = run_bass_kernel_spmd(nc, in_maps, core_ids=list(range(8)),
                                   trace=False)
    global last_results, last_nc
    last_results = res
    last_nc = nc
    parts = [r["out"] for r in res.results]

    # host gather: sum TP halves, add bp and the folded V-bias term
    corr = np.zeros(D, np.float64)
    for g in range(2):
        bv_cat = bv[g * NH:(g + 1) * NH].reshape(NH * DK)
        corr += bv_cat.astype(np.float64) @ Wp.T[g * NH * DK:(g + 1) * NH * DK].astype(np.float64)
    out = np.empty((B, S, D), np.float32)
    for b in range(B):
        out[b] = (parts[2 * b].astype(np.float64)
                  + parts[2 * b + 1].astype(np.float64)
                  + bp.astype(np.float64) + corr).astype(np.float32)
    return out

